# revision 1
# baseline (speedup 1.0000x reference)
"""Trainium2 Bass kernel for fused cosine-distance row merge.

Math (per row i of A, B in [N, D]):
    dot_i   = A[i] . B[i]
    scale_i = max(|A[i]| * |B[i]|, 1e-8)
    w_i     = 1 - dot_i / scale_i
    out[i]  = 0.5 * (w_i * A[i] + (2 - w_i) * B[i])
            = u_i * A[i] + v_i * B[i],  u = 0.5 - 0.5*dot/scale, v = 1 - u

Sharding: pure row-parallel across 8 NeuronCores (N/8 = 2048 rows per core),
no cross-core communication. Each core tiles rows 128-per-partition and
streams [128, t, D] stages (t per the tapered schedule):
  - DVE scalar_tensor_tensor + accum_out: product dump + row dot, one pass
  - ACT Square activation + accum_out: row sum-of-squares for A and B
  - stats ([128, t] tiny ops): u = 0.5 - 0.5*dot/max(|A||B|, EPS)
  - custom DVE lerp op (registered at build time): out = (A-B)*u + B, one pass
The merge stage is software-pipelined one stage behind the stats stage so
DVE always has ready work while ACT accumulates squares. Loads issue on the
SP HWDGE ring; stores on the GPSIMD SWDGE ring so stores never queue behind
later loads. The kernel is DMA-bound: ~24 MB/core over HBM at ~350 GB/s
plus ~16 us of fixed Tile preamble/postamble => ~78-90 us measured.
"""

import numpy as np

import concourse.bacc as bacc
import concourse.mybir as mybir
from concourse.tile import TileContext

N_FULL = 16384
D = 1024
NCORES = 8
ROWS = N_FULL // NCORES  # 2048 rows per core
P = 128  # SBUF partitions
EPS = 1e-8

F32 = mybir.dt.float32

_LERP_NAME = "LERP_MERGE_ANT"


def _get_lerp_op():
    """Register (idempotently) a custom DVE op: out = (in0 - in1)*s0 + in1.

    With in0=A, in1=B, s0=u (per-partition [P,1]) this computes
    u*A + (1-u)*B in a single DVE pass, replacing a tensor_scalar +
    scalar_tensor_tensor pair."""
    from concourse import dve_ops
    from concourse.dve_spec import Spec, Src0, Src1, C0, lower, _has_src1
    from concourse.dve_uop import DveOpSpec

    for op in dve_ops.OPS:
        if op.name == _LERP_NAME:
            return op

    spec = Spec(
        body=(Src0 - Src1) * C0 + Src1,
        reference=lambda in0, in1, s0, s1, imm2: (in0.astype(np.float32) - in1)
        * s0
        + in1,
    )
    row = dve_ops._CUSTOM_DVE_ROW_BASE + len(dve_ops.OPS)
    shas = {}
    for ver in ("v3", "v4"):
        try:
            s = DveOpSpec(
                name=_LERP_NAME,
                opcode=row,
                uops=lower(spec, ver=ver),
                rd1_en=_has_src1(spec),
            )
            shas[ver] = s.sha(ver)
        except Exception:
            pass
    op = dve_ops.DveOp(_LERP_NAME, spec, subdim=False, uops_sha=shas)
    dve_ops.OPS.append(op)
    dve_ops.CUSTOM_DVE_SPECS[_LERP_NAME] = spec
    dve_ops._SUB_OPCODE_FOR_NAME[_LERP_NAME] = row
    return op


def build_program(rows=ROWS, d=D, schedule=None, finalize=True,
                  dump_space="SBUF", io_bufs=4, stat_bufs=3, dump_bufs=1,
                  store_chunk=1, inplace=False, o_bufs=2, layout="strided",
                  store_engine="gpsimd", load_engine_b="sync",
                  dve_dump_space=None, pool_alloc_mode="queue",
                  fine_tail=False, rpp=2):
    """Bass program for one core's [rows, d] shard of A and B.

    `schedule` is a list of per-iteration sub-tile counts (each sub-tile is
    128 rows); tapered ends shorten the pipeline ramp and drain. With
    `inplace` the merge result overwrites the B tile (no separate output
    tile), freeing SBUF for deeper load lookahead."""
    n_sub = rows // P
    if schedule is None:
        schedule = []
        rem = n_sub
        if rem > 8:
            schedule.append(2)
            rem -= 2
        while rem > 6:
            schedule.append(4)
            rem -= 4
        while rem > 0:
            step = min(2, rem)
            schedule.append(step)
            rem -= step
    assert sum(schedule) == n_sub, (schedule, n_sub)
    tmax = max(schedule)

    nc = bacc.Bacc()
    A = nc.declare_dram_parameter("A", [rows, d], F32, isOutput=False)
    B = nc.declare_dram_parameter("B", [rows, d], F32, isOutput=False)
    O = nc.declare_dram_parameter("out", [rows, d], F32, isOutput=True)

    if layout == "contig":
        # partition p owns rows [p*n_sub, (p+1)*n_sub) — a contiguous DRAM
        # block per partition, so every DMA descriptor is a large contiguous
        # read/write. Sub-tile s is the s-th row within each partition's block.
        assert rpp == 1
        Av = A[:].rearrange("(p s) d -> s p d", p=P)
        Bv = B[:].rearrange("(p s) d -> s p d", p=P)
        Ov = O[:].rearrange("(p s) d -> s p d", p=P)
    else:
        # Group g holds rows [g*128*rpp, (g+1)*128*rpp): partition p gets the
        # rpp consecutive rows p*rpp..p*rpp+rpp-1 of the group, concatenated
        # along the free dim. Each DMA descriptor is rpp*4KB contiguous and a
        # group's transfer covers a contiguous DRAM region. rpp=1 is the
        # classic one-row-per-partition sub-tile layout.
        assert all(t % rpp == 0 for t in schedule), (schedule, rpp)
        Av = A[:].rearrange("(g p r) d -> g p (r d)", p=P, r=rpp)
        Bv = B[:].rearrange("(g p r) d -> g p (r d)", p=P, r=rpp)
        Ov = O[:].rearrange("(g p r) d -> g p (r d)", p=P, r=rpp)

    mul = mybir.AluOpType.mult
    add = mybir.AluOpType.add
    Sq = mybir.ActivationFunctionType.Square
    Sqrt = mybir.ActivationFunctionType.Sqrt
    lerp = _get_lerp_op()

    def dram_span(view, s0, t):
        # [P, t//rpp, rpp*d] AP over sub-tiles s0..s0+t-1 (group units inside)
        if layout == "contig":
            ap = view[s0 : s0 + t]  # [t, P, d]
            return ap.rearrange("t p d -> p t d")
        assert s0 % rpp == 0 and t % rpp == 0, (s0, t, rpp)
        ap = view[s0 // rpp : (s0 + t) // rpp]  # [g, P, rpp*d]
        return ap.rearrange("g p f -> p g f")

    def sub_ap(tile3d, j):
        # [P, d] compute slice for sub-tile index j within a stage tile
        return tile3d[:, j // rpp, (j % rpp) * d : (j % rpp + 1) * d]

    with TileContext(nc, pool_alloc_mode=pool_alloc_mode) as tc:
        with (
            tc.tile_pool(name="io", bufs=io_bufs) as io_pool,
            tc.tile_pool(name="opool", bufs=o_bufs) as o_pool,
            tc.tile_pool(name="stat", bufs=stat_bufs) as stat_pool,
            tc.tile_pool(name="dump", bufs=dump_bufs, space=dump_space) as dump_pool,
            tc.tile_pool(
                name="dvedump", bufs=dump_bufs, space=dve_dump_space or dump_space
            ) as dve_dump_pool,
        ):
            store_eng = getattr(nc, store_engine)
            eff_chunk = store_chunk if rpp == 1 else max(store_chunk, rpp)

            def emit_merge(st):
                # lerp + store for a completed stats stage
                a, b, u, m_s0, m_t = st
                o = b if inplace else o_pool.tile(
                    [P, m_t // rpp, rpp * d], F32, tag="o"
                )
                for j in range(m_t):
                    nc.vector._custom_dve(
                        lerp,
                        out=sub_ap(o, j),
                        in0=sub_ap(a, j),
                        in1=sub_ap(b, j),
                        s0=u[:, j : j + 1],
                    )
                    # store as soon as a chunk of sub-tiles is merged
                    if (j + 1) % eff_chunk == 0 or j == m_t - 1:
                        lo = (j // eff_chunk) * eff_chunk
                        store_eng.dma_start(
                            dram_span(Ov, m_s0 + lo, j + 1 - lo),
                            o[:, lo // rpp : (j + rpp) // rpp],
                        )

            pending = None  # software pipeline: merge trails stats by one stage
            s0 = 0
            for stage_idx, t in enumerate(schedule):
                is_last = stage_idx == len(schedule) - 1
                fine = fine_tail and is_last
                a = io_pool.tile([P, t // rpp, rpp * d], F32, tag="a")
                b = io_pool.tile([P, t // rpp, rpp * d], F32, tag="b")
                nc.sync.dma_start(a[:], dram_span(Av, s0, t))
                getattr(nc, load_engine_b).dma_start(b[:], dram_span(Bv, s0, t))

                dot = stat_pool.tile([P, tmax], F32, tag="dot")
                ssa = stat_pool.tile([P, tmax], F32, tag="ssa")
                ssb = stat_pool.tile([P, tmax], F32, tag="ssb")
                dve_dump = dve_dump_pool.tile([P, d], F32, tag="dve")
                act_dump = dump_pool.tile([P, d], F32, tag="act")

                if fine:
                    # flush the pipelined merge before the fine-grained tail
                    if pending is not None:
                        emit_merge(pending)
                        pending = None
                    fo = o_pool.tile([P, t // rpp, rpp * d], F32, tag="o")
                    fsa = stat_pool.tile([P, tmax], F32, tag="sa")
                    fsb = stat_pool.tile([P, tmax], F32, tag="sb")
                    fsc = stat_pool.tile([P, tmax], F32, tag="sc")
                    fr = stat_pool.tile([P, tmax], F32, tag="r")
                    fu = stat_pool.tile([P, tmax], F32, tag="u")

                for j in range(t):
                    # dot[:, j] = sum(A*B) along d; the product goes to a dump
                    # tile. (tensor_tensor_reduce crashes the device on this
                    # runtime; scalar_tensor_tensor with accum_out is the
                    # working single-pass product+row-sum.)
                    nc.vector.scalar_tensor_tensor(
                        dve_dump[:],
                        sub_ap(a, j),
                        1.0,
                        sub_ap(b, j),
                        mul,
                        mul,
                        accum_out=dot[:, j : j + 1],
                    )
                    nc.scalar.activation(
                        act_dump[:], sub_ap(a, j), Sq, accum_out=ssa[:, j : j + 1]
                    )
                    nc.scalar.activation(
                        act_dump[:], sub_ap(b, j), Sq, accum_out=ssb[:, j : j + 1]
                    )
                    if fine:
                        # fine-grained tail: stats + merge + store per sub-tile
                        jj = slice(j, j + 1)
                        nc.scalar.activation(fsa[:, jj], ssa[:, jj], Sqrt)
                        nc.scalar.activation(fsb[:, jj], ssb[:, jj], Sqrt)
                        nc.vector.tensor_mul(fsc[:, jj], fsa[:, jj], fsb[:, jj])
                        nc.vector.tensor_scalar_max(fsc[:, jj], fsc[:, jj], EPS)
                        nc.vector.reciprocal(fr[:, jj], fsc[:, jj])
                        nc.vector.tensor_mul(fu[:, jj], dot[:, jj], fr[:, jj])
                        nc.vector.tensor_scalar(
                            fu[:, jj], fu[:, jj], -0.5, 0.5, mul, add
                        )
                        nc.vector._custom_dve(
                            lerp, out=sub_ap(fo, j), in0=sub_ap(a, j),
                            in1=sub_ap(b, j), s0=fu[:, jj],
                        )
                        if (j + 1) % rpp == 0:
                            store_eng.dma_start(
                                dram_span(Ov, s0 + j + 1 - rpp, rpp),
                                fo[:, j // rpp : j // rpp + 1],
                            )

                if fine:
                    s0 += t
                    continue

                # Previous stage's merge goes here: its u is already computed,
                # so these lerps give DVE ready-to-run work while ACT grinds
                # through this stage's squares.
                if pending is not None:
                    emit_merge(pending)

                # Per-row coefficients, batched over the t sub-tiles.
                # sqrt(ssa)*sqrt(ssb) (not sqrt(ssa*ssb)) keeps the ACT sqrts
                # dependent only on ACT's own accum outputs, so ACT never
                # head-of-line blocks on a DVE tiny op.
                # scale = max(|A||B|, EPS); c = dot/scale; u = 0.5 - 0.5c
                # out = u*A + (1-u)*B = (A-B)*u + B
                sa = stat_pool.tile([P, tmax], F32, tag="sa")
                sb = stat_pool.tile([P, tmax], F32, tag="sb")
                sc = stat_pool.tile([P, tmax], F32, tag="sc")
                r = stat_pool.tile([P, tmax], F32, tag="r")
                u = stat_pool.tile([P, tmax], F32, tag="u")
                nc.scalar.activation(sa[:, :t], ssa[:, :t], Sqrt)
                nc.scalar.activation(sb[:, :t], ssb[:, :t], Sqrt)
                nc.vector.tensor_mul(sc[:, :t], sa[:, :t], sb[:, :t])
                nc.vector.tensor_scalar_max(sc[:, :t], sc[:, :t], EPS)
                nc.vector.reciprocal(r[:, :t], sc[:, :t])
                nc.vector.tensor_mul(u[:, :t], dot[:, :t], r[:, :t])
                nc.vector.tensor_scalar(u[:, :t], u[:, :t], -0.5, 0.5, mul, add)

                pending = (a, b, u, s0, t)
                s0 += t

            if pending is not None:
                emit_merge(pending)

    if finalize:
        nc.finalize()
    return nc


_prog_cache = {}


def _get_program():
    key = (ROWS, D)
    if key not in _prog_cache:
        _prog_cache[key] = build_program()
    return _prog_cache[key]


def kernel(A, B):
    from concourse.bass_utils import run_bass_kernel_spmd

    A = np.asarray(A, dtype=np.float32)
    B = np.asarray(B, dtype=np.float32)
    assert A.shape == (N_FULL, D) and B.shape == (N_FULL, D)

    nc = _get_program()
    in_maps = [
        {
            "A": np.ascontiguousarray(A[i * ROWS : (i + 1) * ROWS]),
            "B": np.ascontiguousarray(B[i * ROWS : (i + 1) * ROWS]),
        }
        for i in range(NCORES)
    ]
    res = run_bass_kernel_spmd(nc, in_maps, list(range(NCORES)))
    return np.concatenate([res.results[i]["out"] for i in range(NCORES)], axis=0)

